# revision 19
# baseline (speedup 1.0000x reference)
"""LSTM encoder (embedding gather + 512-step LSTM) on 8 TRN2 NeuronCores.

Sharding: data-parallel over batch — each of the 8 cores owns 8 of the 64
sequences end-to-end (embedding table and weights replicated), so the
sequential recurrence needs no cross-core communication.

Per-core Bass/Tile kernel:
  Phase 1 (x-phase): indirect-DMA gather of embedding rows, PE-transpose to
    X.T, GEMM xg = X @ W_ih + b -> xg_hbm [S, 32, H] bf16 (dim1 = (gate
    strip j, batch b), strips ordered i, f, o, g).
  Phase 2 (recurrence): hardware For_i loop over S/U iterations, U steps
    unrolled per iteration. Per step:
      - scatter-matmul seeds the gate PSUM [128, 1024] with xg (rows 32j+b)
      - 64 col-strip-packed matmuls accumulate h.T @ W_hh (tile_position,
        k-outer with n interleaved so consecutive MMs alternate PSUM banks)
      - evacuate -> bf16, 8 PE transposes -> hidden-partition gate layout
      - cell update CHUNKED by hidden half: the next step's k=0..3 matmuls
        depend only on hT cols 0..128 (tile subtile deps), so half B's
        activation chain overlaps them. All-sigmoid formulation (g strip of
        W/b pre-scaled x2 on the host; tanh(x) = 2*sigmoid(2x) - 1) merges
        the sigmoid+tanh ACT instructions into one sigmoid per half, and the
        h-mult writes the hT layout directly (hs copy off the critical
        path). Measured 5.14ms vs 5.37ms baseline.
All matmuls are bf16 with fp32 PSUM accumulation; the cell state is fp32.
"""
import sys

if "/opt/trn_rl_repo" not in sys.path:
    sys.path.insert(0, "/opt/trn_rl_repo")

import numpy as np
import ml_dtypes
import concourse.bass as bass
import concourse.tile as tile
from concourse import bacc, mybir
from concourse.masks import make_identity

F32 = mybir.dt.float32
BF16 = mybir.dt.bfloat16
I32 = mybir.dt.int32
P = 128
GATE_PERM = [0, 1, 3, 2]  # strip j -> original gate block (W order: i, f, g, o)

# Problem constants (hardcoded per contest contract)
VOCAB, E, H = 32000, 1024, 1024
B, S = 64, 512
NCORES = 8
BLOC = B // NCORES
U = 16

_program_cache = {}


def build_program(S=S, BLOC=BLOC, E=E, H=H, VOCAB=VOCAB, U=U):
    """x-phase interleaved into the recurrence: iteration iv computes the
    x-gates m-tile for iteration iv+1 in the PE gaps between steps.
    Requires U == 16 (one 128-token m-tile per iteration)."""
    KT = E // P
    KTH = H // P
    GN = 4 * H
    TOK = S * BLOC
    NIT = S // U
    JB = 4 * BLOC
    assert U == 16 and S % U == 0 and TOK // P == NIT

    nc = bacc.Bacc(None, target_bir_lowering=False, debug=False)

    src_idx = nc.dram_tensor("src_idx", [TOK + P, 1], I32, kind="ExternalInput")
    emb = nc.dram_tensor("emb", [VOCAB, E], F32, kind="ExternalInput")
    wih = nc.dram_tensor("wih", [P, KT, GN], BF16, kind="ExternalInput")
    whh = nc.dram_tensor("whh", [P, KTH, GN], BF16, kind="ExternalInput")
    bias = nc.dram_tensor("bias", [GN], F32, kind="ExternalInput")
    scat = nc.dram_tensor("scat", [JB, P], BF16, kind="ExternalInput")
    hs = nc.dram_tensor("hs", [S, P, BLOC * KTH], BF16, kind="ExternalOutput")
    xg_hbm = nc.dram_tensor("xg_hbm", [S + U, JB, H], BF16)

    with tile.TileContext(nc) as tc:
        with tc.tile_pool(name="const", bufs=1) as const, \
             tc.tile_pool(name="rw", bufs=1) as rw, \
             tc.tile_pool(name="state", bufs=1) as state, \
             tc.tile_pool(name="rsb", bufs=3) as rsb, \
             tc.tile_pool(name="rps", bufs=2, space="PSUM") as rps, \
             tc.tile_pool(name="gtps_pool", bufs=1, space="PSUM") as gtps_pool, \
             tc.tile_pool(name="xtp", bufs=2, space="PSUM") as xtp, \
             tc.tile_pool(name="xgp", bufs=1, space="PSUM") as xgp:
            ident = const.tile([P, P], BF16)
            make_identity(nc, ident[:])
            whh_sb = rw.tile([P, KTH, GN], BF16)
            nc.sync.dma_start(out=whh_sb[:], in_=whh[:])
            wih_sb = rw.tile([P, KT, GN], BF16)
            nc.sync.dma_start(out=wih_sb[:], in_=wih[:])
            bias_sb = rw.tile([P, GN], F32)
            nc.sync.dma_start(out=bias_sb[:], in_=bass.AP(
                tensor=bias.ap().tensor, offset=0, ap=[[0, P], [1, GN]]))
            scat_sb = rw.tile([JB, P], BF16)
            nc.sync.dma_start(out=scat_sb[:], in_=scat[:])

            hT = [state.tile([P, KTH * 32], BF16, tag=f"hT{i}", name=f"hT{i}")
                  for i in range(2)]
            cst = [state.tile([P, BLOC * KTH], F32, tag=f"cst{i}", name=f"cst{i}")
                   for i in range(2)]
            nc.vector.memset(hT[0][:], 0.0)
            nc.vector.memset(hT[1][:], 0.0)
            nc.vector.memset(cst[0][:], 0.0)
            xg_it = state.tile([JB, U * H], BF16, tag="xgit")
            hs_it = state.tile([P, U * BLOC * KTH], BF16, tag="hsit")
            idx_sb = state.tile([P, 1], I32, tag="idx")
            xrow = state.tile([P, E], F32, tag="xrow")
            xrow_bf = state.tile([P, E], BF16, tag="xrowbf")
            xt_sb = state.tile([P, KT * P], BF16, tag="xt")

            def x_chunk(u, mt):
                """Emit slice u (0..15) of the x-phase work for m-tile mt
                (int or ScalarValue)."""
                if u == 0:
                    nc.sync.dma_start(out=idx_sb[:],
                                      in_=src_idx[bass.ds(mt * P, P), :])
                    nc.gpsimd.indirect_dma_start(
                        out=xrow[:], out_offset=None, in_=emb[:],
                        in_offset=bass.IndirectOffsetOnAxis(ap=idx_sb[:, :1], axis=0))
                elif u == 1:
                    nc.vector.tensor_copy(out=xrow_bf[:], in_=xrow[:])
                elif 2 <= u <= 5:
                    for c in (2 * (u - 2), 2 * (u - 2) + 1):
                        xt_ps = xtp.tile([P, P], BF16, tag="xtps")
                        nc.tensor.transpose(out=xt_ps[:],
                                            in_=xrow_bf[:, c * P:(c + 1) * P],
                                            identity=ident[:])
                        nc.scalar.copy(out=xt_sb[:, c * P:(c + 1) * P], in_=xt_ps[:])
                elif 6 <= u <= 13:
                    jn = u - 6
                    j, nh = jn // 2, jn % 2
                    xg_ps = xgp.tile([P, 512], F32, tag="xgps")
                    for k in range(KT):
                        nc.tensor.matmul(
                            out=xg_ps[:], lhsT=xt_sb[:, k * P:(k + 1) * P],
                            rhs=wih_sb[:, k, jn * 512:(jn + 1) * 512],
                            start=(k == 0), stop=(k == KT - 1))
                    xgq = rsb.tile([P, 512], BF16, tag="xgq")
                    nc.vector.tensor_tensor(out=xgq[:], in0=xg_ps[:],
                                            in1=bias_sb[:, jn * 512:(jn + 1) * 512],
                                            op=mybir.AluOpType.add)
                    nc.sync.dma_start(
                        out=xg_hbm[bass.ds(mt * (P // BLOC), P // BLOC),
                                   j * BLOC:(j + 1) * BLOC,
                                   nh * 512:(nh + 1) * 512],
                        in_=xgq[:])

            def step(u):
                h_cur, h_new = hT[u % 2], hT[(u + 1) % 2]
                c_cur, c_new = cst[u % 2], cst[(u + 1) % 2]
                g_ps = rps.tile([P, 1024], F32, tag="gps")
                g_sb = rsb.tile([P, 1024], BF16, tag="gsb")
                gt_ps = gtps_pool.tile([P, 1024], BF16, tag="gtps")
                for n in range(2):
                    nc.tensor.matmul(
                        out=g_ps[:, 512 * n:512 * (n + 1)],
                        lhsT=scat_sb[:, :],
                        rhs=xg_it[:, u * H + 512 * n: u * H + 512 * (n + 1)],
                        start=True, stop=True)

                def mm_all():
                    # k-outer, n interleaved: alternating PSUM banks paces
                    # the PE drain pipeline (baseline order)
                    for k in range(KTH):
                        for n in range(2):
                            for j in range(4):
                                nc.tensor.matmul(
                                    out=g_ps[32 * j:32 * (j + 1), 512 * n:512 * (n + 1)],
                                    lhsT=h_cur[:, 32 * k:32 * (k + 1)],
                                    rhs=whh_sb[:, k, j * H + 512 * n: j * H + 512 * (n + 1)],
                                    start=False, stop=(k == KTH - 1),
                                    tile_position=(0, 32 * j),
                                    skip_group_check=True)

                def evac_half(n, eng):
                    eng(out=g_sb[:, 512 * n:512 * n + 512],
                        in_=g_ps[:, 512 * n:512 * n + 512])

                def trans_half(n):
                    for c in range(4 * n, 4 * n + 4):
                        nc.tensor.transpose(out=gt_ps[:, c * P:(c + 1) * P],
                                            in_=g_sb[:, c * P:(c + 1) * P],
                                            identity=ident[:])

                base = gt_ps[:]

                def gt_src(n, j0, nj):
                    # strips j0..j0+nj, k-tiles 4n..4n+4 (hidden half n)
                    return bass.AP(tensor=base.tensor,
                                   offset=base.offset + 32 * j0 + 4 * n * P,
                                   ap=[base.ap[0], [32, nj], [P, 4], [1, BLOC]])

                def act_chunk(c0, cn):
                    # hidden k-tiles [c0, c0+cn): all 4 gate strips present.
                    # g strip is pre-scaled x2 in host prep, so one sigmoid
                    # covers all strips: tanh(x) = 2*sigmoid(2x) - 1.
                    hw = 8 * cn
                    s_all_t = rsb.tile([P, 128], F32, tag="sall", name="s_all_t")
                    s_all = s_all_t[:, 0:4 * hw]
                    nc.scalar.activation(
                        out=s_all.rearrange("p (j c b) -> p j c b", j=4, c=cn),
                        in_=bass.AP(tensor=base.tensor,
                                    offset=base.offset + c0 * P,
                                    ap=[base.ap[0], [32, 4], [P, cn], [1, BLOC]]),
                        func=mybir.ActivationFunctionType.Sigmoid)
                    t_g_t = rsb.tile([P, 32], F32, tag="tg", name="t_g_t")
                    t_g = t_g_t[:, 0:hw]
                    nc.vector.tensor_scalar(out=t_g, in0=s_all[:, 3 * hw:4 * hw],
                                            scalar1=2.0, scalar2=-1.0,
                                            op0=mybir.AluOpType.mult,
                                            op1=mybir.AluOpType.add)
                    fc_t = rsb.tile([P, 32], F32, tag="fc", name="fc_t")
                    fc = fc_t[:, 0:hw]
                    nc.vector.tensor_tensor(out=fc, in0=c_cur[:, 8 * c0:8 * c0 + hw],
                                            in1=s_all[:, hw:2 * hw],
                                            op=mybir.AluOpType.mult)
                    ig_t = rsb.tile([P, 32], F32, tag="ig", name="ig_t")
                    ig = ig_t[:, 0:hw]
                    nc.vector.tensor_tensor(out=ig, in0=t_g, in1=s_all[:, 0:hw],
                                            op=mybir.AluOpType.mult)
                    nc.vector.tensor_tensor(out=c_new[:, 8 * c0:8 * c0 + hw],
                                            in0=fc, in1=ig,
                                            op=mybir.AluOpType.add)
                    sc_t = rsb.tile([P, 32], F32, tag="tc", name="sc_t")
                    sc = sc_t[:, 0:hw]
                    nc.scalar.activation(out=sc, in_=c_new[:, 8 * c0:8 * c0 + hw],
                                         func=mybir.ActivationFunctionType.Sigmoid,
                                         scale=2.0)
                    tc_t = rsb.tile([P, 32], F32, tag="tc2", name="tc_t")
                    tc = tc_t[:, 0:hw]
                    nc.vector.tensor_scalar(out=tc, in0=sc,
                                            scalar1=2.0, scalar2=-1.0,
                                            op0=mybir.AluOpType.mult,
                                            op1=mybir.AluOpType.add)
                    # h -> hT layout directly (critical path for next step's MMs)
                    hT_dst = bass.AP(tensor=h_new.tensor,
                                     offset=h_new[:].offset + 32 * c0,
                                     ap=[h_new[:].ap[0], [32, cn], [1, BLOC]])
                    nc.vector.tensor_tensor(
                        out=hT_dst,
                        in0=tc.rearrange("p (c b) -> p c b", c=cn),
                        in1=s_all[:, 2 * hw:3 * hw].rearrange("p (c b) -> p c b", c=cn),
                        op=mybir.AluOpType.mult)
                    # hs output copy off the critical path
                    hT_src = bass.AP(tensor=h_new.tensor,
                                     offset=h_new[:].offset + 32 * c0,
                                     ap=[h_new[:].ap[0], [32, cn], [1, BLOC]])
                    nc.vector.tensor_copy(
                        out=hs_it[:, u * 64 + 8 * c0: u * 64 + 8 * c0 + hw]
                            .rearrange("p (c b) -> p c b", c=cn),
                        in_=hT_src)

                # act tail in 3 chunks: quarter q0 unlocks next step's
                # k=0..1 matmuls earliest, then q1, then the back half
                mm_all()
                nc.scalar.copy(out=g_sb[:, 0:256], in_=g_ps[:, 0:256])
                for c in range(0, 2):
                    nc.tensor.transpose(out=gt_ps[:, c * P:(c + 1) * P],
                                        in_=g_sb[:, c * P:(c + 1) * P],
                                        identity=ident[:])
                nc.vector.tensor_copy(out=g_sb[:, 256:512], in_=g_ps[:, 256:512])
                for c in range(2, 4):
                    nc.tensor.transpose(out=gt_ps[:, c * P:(c + 1) * P],
                                        in_=g_sb[:, c * P:(c + 1) * P],
                                        identity=ident[:])
                nc.vector.tensor_copy(out=g_sb[:, 512:1024], in_=g_ps[:, 512:1024])
                for c in range(4, 8):
                    nc.tensor.transpose(out=gt_ps[:, c * P:(c + 1) * P],
                                        in_=g_sb[:, c * P:(c + 1) * P],
                                        identity=ident[:])
                act_chunk(0, 2)
                act_chunk(2, 2)
                act_chunk(4, 4)
            # prologue: x m-tile 0
            for u in range(U):
                x_chunk(u, 0)

            with tc.For_i(0, NIT, 1) as iv:
                nc.sync.dma_start(
                    out=xg_it[:].rearrange("p (t h) -> p t h", t=U),
                    in_=xg_hbm[bass.ds(iv * U, U), :, :].rearrange("t p h -> p t h"))
                for u in range(U):
                    step(u)
                    x_chunk(u, iv + 1)
                nc.sync.dma_start(
                    out=hs[bass.ds(iv * U, U), :, :].rearrange("t p c -> p t c"),
                    in_=hs_it[:].rearrange("p (t c) -> p t c", t=U))

    nc.compile()
    return nc


def _prep_inputs(source, embedding, W_ih, W_hh, b, core, n_cores=NCORES):
    src_k = np.asarray(source[core * BLOC:(core + 1) * BLOC, :], dtype=np.int32)
    idx = np.ascontiguousarray(src_k.T.reshape(-1, 1))  # (t-major, b)
    idx = np.concatenate([idx, np.zeros((P, 1), np.int32)], axis=0)  # slack m-tile

    def prep_w(W, K):
        Wr = np.asarray(W, np.float32).reshape(K // P, P, 4, H)[:, :, GATE_PERM, :]
        Wr = Wr.copy()
        Wr[:, :, 3, :] *= 2.0  # g strip: tanh(x) = 2*sigmoid(2x) - 1
        return np.ascontiguousarray(
            Wr.transpose(1, 0, 2, 3).reshape(P, K // P, 4 * H)).astype(ml_dtypes.bfloat16)

    bias_sc = np.asarray(b, np.float32).reshape(4, H)[GATE_PERM].copy()
    bias_sc[3] *= 2.0  # g strip pre-scale
    bias_dev = np.ascontiguousarray(bias_sc.reshape(4 * H))
    JB = 4 * BLOC
    scat = np.zeros((JB, P), np.float32)
    for j in range(4):
        for bb in range(BLOC):
            scat[j * BLOC + bb, 32 * j + bb] = 1.0
    return {
        "src_idx": idx,
        "emb": np.asarray(embedding, np.float32),
        "wih": prep_w(W_ih, E),
        "whh": prep_w(W_hh, H),
        "bias": bias_dev,
        "scat": scat.astype(ml_dtypes.bfloat16),
    }


def _unpack_output(hs_dev):
    KTH = H // P
    a = np.asarray(hs_dev, dtype=np.float32).reshape(S, P, KTH, BLOC)
    return np.ascontiguousarray(a.transpose(3, 0, 2, 1)).reshape(BLOC, S, H)


# Weight prep is deterministic; cache per-core input maps keyed on id of arrays.
def _get_program():
    if "nc" not in _program_cache:
        _program_cache["nc"] = build_program()
    return _program_cache["nc"]


def kernel(source, embedding, W_ih, W_hh, b):
    """Full inputs in, full output out. Shards batch over 8 NeuronCores."""
    from concourse import bass2jax

    source = np.asarray(source)
    embedding = np.asarray(embedding, np.float32)
    W_ih = np.asarray(W_ih, np.float32)
    W_hh = np.asarray(W_hh, np.float32)
    b = np.asarray(b, np.float32)

    nc = _get_program()
    in_maps = [_prep_inputs(source, embedding, W_ih, W_hh, b, core=k)
               for k in range(NCORES)]
    res = bass2jax.run_bass_via_pjrt(nc, in_maps, n_cores=NCORES)
    out = np.concatenate([_unpack_output(res[k]["hs"]) for k in range(NCORES)],
                         axis=0)
    return out.astype(np.float32)



# revision 20
# speedup vs baseline: 1.6839x; 1.6839x over previous
"""LSTM encoder (embedding gather + 512-step LSTM) on 8 TRN2 NeuronCores.

Sharding: data-parallel over batch — each of the 8 cores owns 8 of the 64
sequences end-to-end (embedding table and weights replicated), so the
sequential recurrence needs no cross-core communication.

Per-core Bass/Tile kernel:
  Phase 1 (x-phase): indirect-DMA gather of embedding rows, PE-transpose to
    X.T, GEMM xg = X @ W_ih + b -> xg_hbm [S, 32, H] bf16 (dim1 = (gate
    strip j, batch b), strips ordered i, f, o, g).
  Phase 2 (recurrence): hardware For_i loop over S/U iterations, U steps
    unrolled per iteration. Per step:
      - scatter-matmul seeds the gate PSUM [128, 1024] with xg (rows 32j+b)
      - 64 col-strip-packed matmuls accumulate h.T @ W_hh (tile_position,
        k-outer with n interleaved so consecutive MMs alternate PSUM banks)
      - evacuate -> bf16, 8 PE transposes -> hidden-partition gate layout
      - cell update CHUNKED by hidden half: the next step's k=0..3 matmuls
        depend only on hT cols 0..128 (tile subtile deps), so half B's
        activation chain overlaps them. All-sigmoid formulation (g strip of
        W/b pre-scaled x2 on the host; tanh(x) = 2*sigmoid(2x) - 1) merges
        the sigmoid+tanh ACT instructions into one sigmoid per half, and the
        h-mult writes the hT layout directly (hs copy off the critical
        path). Measured 5.14ms vs 5.37ms baseline.
All matmuls are bf16 with fp32 PSUM accumulation; the cell state is fp32.
"""
import sys

if "/opt/trn_rl_repo" not in sys.path:
    sys.path.insert(0, "/opt/trn_rl_repo")

import numpy as np
import ml_dtypes
import concourse.bass as bass
import concourse.tile as tile
from concourse import bacc, mybir
from concourse.masks import make_identity

F32 = mybir.dt.float32
BF16 = mybir.dt.bfloat16
I32 = mybir.dt.int32
P = 128
GATE_PERM = [0, 1, 3, 2]  # strip j -> original gate block (W order: i, f, g, o)

# Problem constants (hardcoded per contest contract)
VOCAB, E, H = 32000, 1024, 1024
B, S = 64, 512
NCORES = 8
BLOC = B // NCORES
U = 16

_program_cache = {}


def build_program(S=S, BLOC=BLOC, E=E, H=H, VOCAB=VOCAB, U=U):
    """x-phase interleaved into the recurrence: iteration iv computes the
    x-gates m-tile for iteration iv+1 in the PE gaps between steps.
    Requires U == 16 (one 128-token m-tile per iteration)."""
    KT = E // P
    KTH = H // P
    GN = 4 * H
    TOK = S * BLOC
    NIT = S // U
    JB = 4 * BLOC
    assert U == 16 and S % U == 0 and TOK // P == NIT

    nc = bacc.Bacc(None, target_bir_lowering=False, debug=False)

    src_idx = nc.dram_tensor("src_idx", [TOK + P, 1], I32, kind="ExternalInput")
    emb = nc.dram_tensor("emb", [VOCAB, E], F32, kind="ExternalInput")
    wih = nc.dram_tensor("wih", [P, KT, GN], BF16, kind="ExternalInput")
    whh = nc.dram_tensor("whh", [P, KTH, GN], BF16, kind="ExternalInput")
    bias = nc.dram_tensor("bias", [GN], F32, kind="ExternalInput")
    scat = nc.dram_tensor("scat", [JB, P], BF16, kind="ExternalInput")
    hs = nc.dram_tensor("hs", [S, P, BLOC * KTH], BF16, kind="ExternalOutput")
    xg_hbm = nc.dram_tensor("xg_hbm", [S + U, JB, H], BF16)

    with tile.TileContext(nc) as tc:
        with tc.tile_pool(name="const", bufs=1) as const, \
             tc.tile_pool(name="rw", bufs=1) as rw, \
             tc.tile_pool(name="state", bufs=1) as state, \
             tc.tile_pool(name="rsb", bufs=2) as rsb, \
             tc.tile_pool(name="rps", bufs=2, space="PSUM") as rps, \
             tc.tile_pool(name="gtps_pool", bufs=1, space="PSUM") as gtps_pool, \
             tc.tile_pool(name="xtp", bufs=2, space="PSUM") as xtp, \
             tc.tile_pool(name="xgp", bufs=1, space="PSUM") as xgp:
            ident = const.tile([P, P], BF16)
            make_identity(nc, ident[:])
            whh_sb = rw.tile([P, KTH, GN], BF16)
            nc.sync.dma_start(out=whh_sb[:], in_=whh[:])
            wih_sb = rw.tile([P, KT, GN], BF16)
            nc.sync.dma_start(out=wih_sb[:], in_=wih[:])
            bias_sb = rw.tile([P, GN], F32)
            nc.sync.dma_start(out=bias_sb[:], in_=bass.AP(
                tensor=bias.ap().tensor, offset=0, ap=[[0, P], [1, GN]]))
            scat_sb = rw.tile([JB, P], BF16)
            nc.sync.dma_start(out=scat_sb[:], in_=scat[:])

            hT = [state.tile([P, KTH * 32], BF16, tag=f"hT{i}", name=f"hT{i}")
                  for i in range(2)]
            cst = [state.tile([P, BLOC * KTH], F32, tag=f"cst{i}", name=f"cst{i}")
                   for i in range(2)]
            nc.vector.memset(hT[0][:], 0.0)
            nc.vector.memset(hT[1][:], 0.0)
            nc.vector.memset(cst[0][:], 0.0)
            xg_it = state.tile([JB, U * H], BF16, tag="xgit")
            hs_it = state.tile([P, U * BLOC * KTH], BF16, tag="hsit")
            idx_sb = state.tile([P, 1], I32, tag="idx")
            xrow = state.tile([P, E], F32, tag="xrow")
            xrow_bf = state.tile([P, E], BF16, tag="xrowbf")
            xt_sb = state.tile([P, KT * P], BF16, tag="xt")

            def x_chunk(u, mt):
                """Emit slice u (0..15) of the x-phase work for m-tile mt
                (int or ScalarValue)."""
                if u == 0:
                    nc.sync.dma_start(out=idx_sb[:],
                                      in_=src_idx[bass.ds(mt * P, P), :])
                    nc.gpsimd.indirect_dma_start(
                        out=xrow[:], out_offset=None, in_=emb[:],
                        in_offset=bass.IndirectOffsetOnAxis(ap=idx_sb[:, :1], axis=0))
                elif u == 1:
                    nc.vector.tensor_copy(out=xrow_bf[:], in_=xrow[:])
                elif 2 <= u <= 5:
                    for c in (2 * (u - 2), 2 * (u - 2) + 1):
                        xt_ps = xtp.tile([P, P], BF16, tag="xtps")
                        nc.tensor.transpose(out=xt_ps[:],
                                            in_=xrow_bf[:, c * P:(c + 1) * P],
                                            identity=ident[:])
                        nc.scalar.copy(out=xt_sb[:, c * P:(c + 1) * P], in_=xt_ps[:])
                elif 6 <= u <= 13:
                    jn = u - 6
                    j, nh = jn // 2, jn % 2
                    xg_ps = xgp.tile([P, 512], F32, tag="xgps")
                    for k in range(KT):
                        nc.tensor.matmul(
                            out=xg_ps[:], lhsT=xt_sb[:, k * P:(k + 1) * P],
                            rhs=wih_sb[:, k, jn * 512:(jn + 1) * 512],
                            start=(k == 0), stop=(k == KT - 1))
                    xgq = rsb.tile([P, 512], BF16, tag="xgq")
                    nc.vector.tensor_tensor(out=xgq[:], in0=xg_ps[:],
                                            in1=bias_sb[:, jn * 512:(jn + 1) * 512],
                                            op=mybir.AluOpType.add)
                    nc.sync.dma_start(
                        out=xg_hbm[bass.ds(mt * (P // BLOC), P // BLOC),
                                   j * BLOC:(j + 1) * BLOC,
                                   nh * 512:(nh + 1) * 512],
                        in_=xgq[:])

            def step(u):
                h_cur, h_new = hT[u % 2], hT[(u + 1) % 2]
                c_cur, c_new = cst[u % 2], cst[(u + 1) % 2]
                g_ps = rps.tile([P, 1024], F32, tag="gps")
                g_sb = rsb.tile([P, 1024], BF16, tag="gsb")
                gt_ps = gtps_pool.tile([P, 1024], BF16, tag="gtps")
                for n in range(2):
                    nc.tensor.matmul(
                        out=g_ps[:, 512 * n:512 * (n + 1)],
                        lhsT=scat_sb[:, :],
                        rhs=xg_it[:, u * H + 512 * n: u * H + 512 * (n + 1)],
                        start=True, stop=True)

                def mm_all():
                    # k-outer, n interleaved: alternating PSUM banks paces
                    # the PE drain pipeline (baseline order)
                    for k in range(KTH):
                        for n in range(2):
                            for j in range(4):
                                nc.tensor.matmul(
                                    out=g_ps[32 * j:32 * (j + 1), 512 * n:512 * (n + 1)],
                                    lhsT=h_cur[:, 32 * k:32 * (k + 1)],
                                    rhs=whh_sb[:, k, j * H + 512 * n: j * H + 512 * (n + 1)],
                                    start=False, stop=(k == KTH - 1),
                                    tile_position=(0, 32 * j),
                                    skip_group_check=True)

                def evac_half(n, eng):
                    eng(out=g_sb[:, 512 * n:512 * n + 512],
                        in_=g_ps[:, 512 * n:512 * n + 512])

                def trans_half(n):
                    for c in range(4 * n, 4 * n + 4):
                        nc.tensor.transpose(out=gt_ps[:, c * P:(c + 1) * P],
                                            in_=g_sb[:, c * P:(c + 1) * P],
                                            identity=ident[:])

                base = gt_ps[:]

                def gt_src(n, j0, nj):
                    # strips j0..j0+nj, k-tiles 4n..4n+4 (hidden half n)
                    return bass.AP(tensor=base.tensor,
                                   offset=base.offset + 32 * j0 + 4 * n * P,
                                   ap=[base.ap[0], [32, nj], [P, 4], [1, BLOC]])

                def act_half(n):
                    # hidden units [512n, 512n+512): all 4 gate strips present.
                    # g strip is pre-scaled x2 in host prep, so one sigmoid
                    # covers all strips: tanh(x) = 2*sigmoid(2x) - 1.
                    hw = 32  # h/c cols per half (4 ktiles x 8 batch)
                    s_all_t = rsb.tile([P, 128], F32, tag="sall", name="s_all_t")
                    s_all = s_all_t[:]
                    nc.scalar.activation(
                        out=s_all.rearrange("p (j c b) -> p j c b", j=4, c=4),
                        in_=gt_src(n, 0, 4),
                        func=mybir.ActivationFunctionType.Sigmoid)
                    t_g_t = rsb.tile([P, 32], F32, tag="tg", name="t_g_t")
                    t_g = t_g_t[:]
                    nc.vector.tensor_scalar(out=t_g, in0=s_all[:, 96:128],
                                            scalar1=2.0, scalar2=-1.0,
                                            op0=mybir.AluOpType.mult,
                                            op1=mybir.AluOpType.add)
                    fc_t = rsb.tile([P, 32], F32, tag="fc", name="fc_t")
                    fc = fc_t[:]
                    nc.vector.tensor_tensor(out=fc, in0=c_cur[:, hw * n:hw * n + hw],
                                            in1=s_all[:, 32:64],
                                            op=mybir.AluOpType.mult)
                    ig_t = rsb.tile([P, 32], F32, tag="ig", name="ig_t")
                    ig = ig_t[:]
                    nc.vector.tensor_tensor(out=ig, in0=t_g, in1=s_all[:, 0:32],
                                            op=mybir.AluOpType.mult)
                    nc.vector.tensor_tensor(out=c_new[:, hw * n:hw * n + hw],
                                            in0=fc, in1=ig,
                                            op=mybir.AluOpType.add)
                    sc_t = rsb.tile([P, 32], F32, tag="tc", name="sc_t")
                    sc = sc_t[:]
                    nc.scalar.activation(out=sc, in_=c_new[:, hw * n:hw * n + hw],
                                         func=mybir.ActivationFunctionType.Sigmoid,
                                         scale=2.0)
                    tc_t = rsb.tile([P, 32], F32, tag="tc2", name="tc_t")
                    tc = tc_t[:]
                    nc.vector.tensor_scalar(out=tc, in0=sc,
                                            scalar1=2.0, scalar2=-1.0,
                                            op0=mybir.AluOpType.mult,
                                            op1=mybir.AluOpType.add)
                    # h -> hT layout directly (critical path for next step's MMs)
                    hT_dst = bass.AP(tensor=h_new.tensor,
                                     offset=h_new[:].offset + 32 * 4 * n,
                                     ap=[h_new[:].ap[0], [32, 4], [1, BLOC]])
                    nc.vector.tensor_tensor(
                        out=hT_dst,
                        in0=tc.rearrange("p (c b) -> p c b", c=4),
                        in1=s_all[:, 64:96].rearrange("p (c b) -> p c b", c=4),
                        op=mybir.AluOpType.mult)
                    # hs output copy off the critical path
                    hT_src = bass.AP(tensor=h_new.tensor,
                                     offset=h_new[:].offset + 32 * 4 * n,
                                     ap=[h_new[:].ap[0], [32, 4], [1, BLOC]])
                    nc.vector.tensor_copy(
                        out=hs_it[:, u * 64 + hw * n: u * 64 + hw * n + hw]
                            .rearrange("p (c b) -> p c b", c=4),
                        in_=hT_src)

                # act tail chunked by hidden half: next step's k=0..3 matmuls
                # depend only on hT half A, so half B's act chain overlaps them
                mm_all()
                evac_half(0, nc.scalar.copy)
                trans_half(0)
                evac_half(1, nc.vector.tensor_copy)
                trans_half(1)
                act_half(0)
                act_half(1)

            # prologue: x m-tile 0
            for u in range(U):
                x_chunk(u, 0)

            with tc.For_i(0, NIT, 1) as iv:
                nc.sync.dma_start(
                    out=xg_it[:].rearrange("p (t h) -> p t h", t=U),
                    in_=xg_hbm[bass.ds(iv * U, U), :, :].rearrange("t p h -> p t h"))
                for u in range(U):
                    step(u)
                    x_chunk(u, iv + 1)
                nc.sync.dma_start(
                    out=hs[bass.ds(iv * U, U), :, :].rearrange("t p c -> p t c"),
                    in_=hs_it[:].rearrange("p (t c) -> p t c", t=U))

    nc.compile()
    return nc


def _prep_inputs(source, embedding, W_ih, W_hh, b, core, n_cores=NCORES):
    src_k = np.asarray(source[core * BLOC:(core + 1) * BLOC, :], dtype=np.int32)
    idx = np.ascontiguousarray(src_k.T.reshape(-1, 1))  # (t-major, b)
    idx = np.concatenate([idx, np.zeros((P, 1), np.int32)], axis=0)  # slack m-tile

    def prep_w(W, K):
        Wr = np.asarray(W, np.float32).reshape(K // P, P, 4, H)[:, :, GATE_PERM, :]
        Wr = Wr.copy()
        Wr[:, :, 3, :] *= 2.0  # g strip: tanh(x) = 2*sigmoid(2x) - 1
        return np.ascontiguousarray(
            Wr.transpose(1, 0, 2, 3).reshape(P, K // P, 4 * H)).astype(ml_dtypes.bfloat16)

    bias_sc = np.asarray(b, np.float32).reshape(4, H)[GATE_PERM].copy()
    bias_sc[3] *= 2.0  # g strip pre-scale
    bias_dev = np.ascontiguousarray(bias_sc.reshape(4 * H))
    JB = 4 * BLOC
    scat = np.zeros((JB, P), np.float32)
    for j in range(4):
        for bb in range(BLOC):
            scat[j * BLOC + bb, 32 * j + bb] = 1.0
    return {
        "src_idx": idx,
        "emb": np.asarray(embedding, np.float32),
        "wih": prep_w(W_ih, E),
        "whh": prep_w(W_hh, H),
        "bias": bias_dev,
        "scat": scat.astype(ml_dtypes.bfloat16),
    }


def _unpack_output(hs_dev):
    KTH = H // P
    a = np.asarray(hs_dev, dtype=np.float32).reshape(S, P, KTH, BLOC)
    return np.ascontiguousarray(a.transpose(3, 0, 2, 1)).reshape(BLOC, S, H)


# Weight prep is deterministic; cache per-core input maps keyed on id of arrays.
def _get_program():
    if "nc" not in _program_cache:
        _program_cache["nc"] = build_program()
    return _program_cache["nc"]


def kernel(source, embedding, W_ih, W_hh, b):
    """Full inputs in, full output out. Shards batch over 8 NeuronCores."""
    from concourse import bass2jax

    source = np.asarray(source)
    embedding = np.asarray(embedding, np.float32)
    W_ih = np.asarray(W_ih, np.float32)
    W_hh = np.asarray(W_hh, np.float32)
    b = np.asarray(b, np.float32)

    nc = _get_program()
    in_maps = [_prep_inputs(source, embedding, W_ih, W_hh, b, core=k)
               for k in range(NCORES)]
    res = bass2jax.run_bass_via_pjrt(nc, in_maps, n_cores=NCORES)
    out = np.concatenate([_unpack_output(res[k]["hs"]) for k in range(NCORES)],
                         axis=0)
    return out.astype(np.float32)

